# revision 33
# baseline (speedup 1.0000x reference)
"""Trainium2 Bass kernel for nn_HaarDecomposition2D.

The reference computes a 9-level redundant "diagonal Haar" decomposition of a
(8,3,512,512) image batch, emitting per-level full-resolution detail images
plus the final low-pass, concatenated to (8,30,512,512).

Algebraic structure (verified bit-exact vs the reference):
the one-level transform is a projection — its low-pass output is a fixed
point of the level map, so every detail level >= 2 is exactly zero and
low_9 == low_1.  The kernel therefore computes det_1 and low_1 only.
Channels 3..26 are exactly zero; run_bass_kernel_spmd's contract pre-zeros
ExternalOutput buffers on both the native path (out_maps) and the axon/PJRT
path (donated zero buffers), so the kernel does not write them.  kernel()
additionally re-asserts those zeros host-side.

Sharding: pure batch data-parallel, batch item b -> NeuronCore b (8 cores).

Math (per 4-row group, rows r0..r3 = 4I..4I+3; P_m = XOR-m column
permutation within 4-blocks, P_m(t)[j] = t[(j&~3)|((j&3)^m)]):

  EI = X[r0] + P1(X[r1])        OI = X[r2] + P1(X[r3])
  L0 = 0.25*(EI + P2(OI))       D0 = 0.25*(EI - P2(OI))
  low[4I+r] = P_r(L0)           det[4I+r] = P_r(D0)     for r = 0..3

(The r-independence follows from a_r ^ b_r == 2 for all output rows in the
original mask pairs (0,2),(1,3),(2,0),(3,1).)  P2(OI) is built directly
(pair-split so all APs stay <=3D), so L0/D0 are fully contiguous fused ops.
The vector engine does the five 2-input ops + scale (~4.3us/channel); the
remaining work is eight permuted copies per channel — 1-input ops that the
scalar (ACT) engine runs at full rate (~4.9us/channel), keeping it ahead of
the 5.06us/channel store-drain cadence.  All DMA (2 loads + 6 stores) is
issued on the sync HWDGE ring, loads first, so the ring streams phase-clean
(reads ~359 GB/s, then writes ~396 GB/s; mixing read/write phases measured
strictly worse) — the kernel is HBM-streaming-bound: ~6.9us framework
preamble + ~23.5us streaming + ~2.4us epilogue ≈ 33-36us.

Per-core layout: each 512x512 channel is an SBUF tile [128, 2048] where
partition I holds image rows 4I..4I+3 (row 4I+q at columns 512q..512q+511).
"""

import sys

if "/opt/trn_rl_repo" not in sys.path:
    sys.path.insert(0, "/opt/trn_rl_repo")

import numpy as np

_NCORES = 8
_C = 3
_H = 512
_W = 512
_OC = 30  # 9 detail levels * 3 channels + 3 low-pass channels

_nc_cache = {}


def _build_nc():
    """Build the per-core Bass program: in x[3,512,512] -> out[30,512,512]."""
    import concourse.bacc as bacc
    import concourse.bass as bass
    import concourse.mybir as mybir
    from concourse.tile import TileContext

    fp32 = mybir.dt.float32
    A = mybir.AluOpType

    nc = bacc.Bacc("TRN2", target_bir_lowering=False, debug=False,
                   enable_asserts=False, enable_partition_id=False,
                   monotonic_sem_count=0)

    xt = nc.dram_tensor("x", [_C, _H, _W], fp32, kind="ExternalInput")
    ot = nc.dram_tensor("out", [_OC, _H, _W], fp32, kind="ExternalOutput")

    def img4(ap):
        # [512,512] image -> [128, 2048]: partition I holds rows 4I..4I+3
        return ap.rearrange("(p q) w -> p (q w)", q=4)

    def view(tile, off, free_ap):
        # free-dim view of a [128, W] tile: keep the partition dim, replace
        # the free dims; offset in elements from the tile base.
        base = tile[:]
        return bass.AP(tile.tensor, base.offset + off,
                       [list(base.ap[0])] + free_ap)

    P1 = [[2, 256], [-1, 2]]     # j -> j^1 (offset +1)
    P3 = [[4, 128], [-1, 4]]     # j -> j^3 (offset +3)
    PAIR = [[4, 128], [1, 2]]    # elements {4t+off, 4t+off+1}

    with TileContext(nc) as tc:
        with tc.tile_pool(name="img", bufs=3) as img_pool, \
             tc.tile_pool(name="outp", bufs=3) as out_pool, \
             tc.tile_pool(name="eo", bufs=2) as eo_pool:

            v = nc.vector
            act = nc.scalar

            X = [None] * _C
            L = [None] * _C
            D = [None] * _C

            def load(c):
                # Each channel loads as two 512 KiB halves with separate
                # completion sems: rows {4I,4I+1} then {4I+2,4I+3}.  EI only
                # needs the first half, and the DMA completion receipt lags
                # the last byte by ~2us — finer sem granularity starts each
                # channel's compute chain ~1.4us earlier, keeping ACT (and
                # so the store dispatches) ahead of the ring.
                X[c] = (img_pool.tile([128, 2048], fp32, tag="X",
                                      name=f"X{c}"), 0)
                src = img4(xt[c])
                nc.sync.dma_start(out=X[c][0][:, 0:1024], in_=src[:, 0:1024])
                nc.sync.dma_start(out=X[c][0][:, 1024:2048],
                                  in_=src[:, 1024:2048])

            def compute(c):
                EI = eo_pool.tile([128, 512], fp32, tag="EI", name=f"EI{c}")
                OIu = eo_pool.tile([128, 512], fp32, tag="OIu", name=f"OIu{c}")
                OI2 = eo_pool.tile([128, 512], fp32, tag="OI2", name=f"OI2{c}")
                Xt, xb = X[c]
                # EI = X_r0 + P1(X_r1)
                v.tensor_tensor(out=EI[:], in0=Xt[:, xb:xb + 512],
                                in1=view(Xt, xb + 512 + 1, P1), op=A.add)
                # OIu = P2(OI) = X_r2[j^2] + X_r3[j^3], built pair-split so
                # every AP stays 3D; then OI2 = 0.25*OIu.
                for h in (0, 2):
                    v.tensor_tensor(
                        out=view(OIu, h, PAIR),
                        in0=view(Xt, xb + 1024 + (h ^ 2), PAIR),
                        in1=view(Xt, xb + 1536 + (h ^ 2) + 1,
                                 [[4, 128], [-1, 2]]),
                        op=A.add)
                v.tensor_scalar_mul(OI2[:], OIu[:], 0.25)

                L[c] = out_pool.tile([128, 2048], fp32, tag="L", name=f"L{c}")
                D[c] = out_pool.tile([128, 2048], fp32, tag="D", name=f"D{c}")
                # L0/D0 into the r=0 block — fully contiguous fused ops.
                v.scalar_tensor_tensor(out=L[c][:, 0:512], in0=EI[:],
                                       scalar=0.25, in1=OI2[:],
                                       op0=A.mult, op1=A.add)
                v.scalar_tensor_tensor(out=D[c][:, 0:512], in0=EI[:],
                                       scalar=0.25, in1=OI2[:],
                                       op0=A.mult, op1=A.subtract)

                # low[r] = P_r(L0), det[r] = P_r(D0): permuted copies, all on
                # ACT (runs them at full rate; 4.9us/channel sustains the
                # 5.06us/channel store cadence).  L copies first so the L
                # store dispatches early.
                for t in (L[c], D[c]):
                    act.copy(t[:, 512:1024], view(t, 1, P1))
                    act.copy(view(t, 1024 + 0, PAIR), view(t, 2, PAIR))
                    act.copy(view(t, 1024 + 2, PAIR), view(t, 0, PAIR))
                    act.copy(t[:, 1536:2048], view(t, 3, P3))

            def store(c):
                nc.sync.dma_start(out=img4(ot[27 + c]), in_=L[c][:])
                nc.sync.dma_start(out=img4(ot[c]), in_=D[c][:])

            load(0)
            load(1)
            load(2)
            compute(0)
            store(0)
            compute(1)
            store(1)
            compute(2)
            store(2)

    nc.finalize()
    return nc


def _get_nc():
    if "nc" not in _nc_cache:
        _nc_cache["nc"] = _build_nc()
    return _nc_cache["nc"]


def run_spmd(x, **kwargs):
    """Run the SPMD kernel on 8 cores; returns (stacked_output, BassKernelResults)."""
    from concourse.bass_utils import run_bass_kernel_spmd

    x = np.ascontiguousarray(np.asarray(x, dtype=np.float32))
    assert x.shape == (_NCORES, _C, _H, _W), x.shape
    nc = _get_nc()
    in_maps = [{"x": np.ascontiguousarray(x[b])} for b in range(_NCORES)]
    res = run_bass_kernel_spmd(nc, in_maps, core_ids=list(range(_NCORES)),
                               **kwargs)
    out = np.stack([res.results[b]["out"] for b in range(_NCORES)], axis=0)
    # channels 3..26 are mathematically zero; the device relies on the
    # pre-zeroed output contract — re-assert host-side for safety.
    out[:, 3:27] = 0.0
    return out, res


def kernel(x):
    out, _ = run_spmd(x)
    return out


# revision 34
# speedup vs baseline: 1.1474x; 1.1474x over previous
"""Trainium2 Bass kernel for nn_HaarDecomposition2D.

The reference computes a 9-level redundant "diagonal Haar" decomposition of a
(8,3,512,512) image batch, emitting per-level full-resolution detail images
plus the final low-pass, concatenated to (8,30,512,512).

Algebraic structure (verified bit-exact vs the reference):
the one-level transform is a projection — its low-pass output is a fixed
point of the level map, so every detail level >= 2 is exactly zero and
low_9 == low_1.  The kernel therefore computes det_1 and low_1 only.
Channels 3..26 are exactly zero; run_bass_kernel_spmd's contract pre-zeros
ExternalOutput buffers on both the native path (out_maps) and the axon/PJRT
path (donated zero buffers), so the kernel does not write them.  kernel()
additionally re-asserts those zeros host-side.

Sharding: pure batch data-parallel, batch item b -> NeuronCore b (8 cores).

Math (per 4-row group, rows r0..r3 = 4I..4I+3; P_m = XOR-m column
permutation within 4-blocks, P_m(t)[j] = t[(j&~3)|((j&3)^m)]):

  EI = X[r0] + P1(X[r1])        OI = X[r2] + P1(X[r3])
  L0 = 0.25*(EI + P2(OI))       D0 = 0.25*(EI - P2(OI))
  low[4I+r] = P_r(L0)           det[4I+r] = P_r(D0)     for r = 0..3

(The r-independence follows from a_r ^ b_r == 2 for all output rows in the
original mask pairs (0,2),(1,3),(2,0),(3,1).)  P2(OI) is built directly
(pair-split so all APs stay <=3D), so L0/D0 are fully contiguous fused ops.
The vector engine does the five 2-input ops + scale (~4.3us/channel); the
remaining work is eight permuted copies per channel — 1-input ops that the
scalar (ACT) engine runs at full rate (~4.9us/channel), keeping it ahead of
the 5.06us/channel store-drain cadence.  All DMA (6 half-channel loads +
6 full stores) is issued on the sync HWDGE ring, loads first, so the ring
streams phase-clean (reads ~359 GB/s, then writes ~396 GB/s; mixing
read/write phases measured strictly worse).  Loads are split into 512 KiB
halves because the DMA completion receipt lags the last byte by ~2us and
EI only needs the first half — per-half sems start each channel's compute
chain ~1.4-2.7us earlier, keeping store dispatches ahead of the ring.
The kernel is HBM-streaming-bound: ~6.9us framework preamble + ~23.5us
streaming + ~2.4us epilogue ≈ 33-36us.

Per-core layout: each 512x512 channel is an SBUF tile [128, 2048] where
partition I holds image rows 4I..4I+3 (row 4I+q at columns 512q..512q+511).
"""

import sys

if "/opt/trn_rl_repo" not in sys.path:
    sys.path.insert(0, "/opt/trn_rl_repo")

import numpy as np

_NCORES = 8
_C = 3
_H = 512
_W = 512
_OC = 30  # 9 detail levels * 3 channels + 3 low-pass channels

_nc_cache = {}


def _build_nc():
    """Build the per-core Bass program: in x[3,512,512] -> out[30,512,512]."""
    import concourse.bacc as bacc
    import concourse.bass as bass
    import concourse.mybir as mybir
    from concourse.tile import TileContext

    fp32 = mybir.dt.float32
    A = mybir.AluOpType

    nc = bacc.Bacc("TRN2", target_bir_lowering=False, debug=False,
                   enable_asserts=False, enable_partition_id=False,
                   monotonic_sem_count=0)

    xt = nc.dram_tensor("x", [_C, _H, _W], fp32, kind="ExternalInput")
    ot = nc.dram_tensor("out", [_OC, _H, _W], fp32, kind="ExternalOutput")

    def img4(ap):
        # [512,512] image -> [128, 2048]: partition I holds rows 4I..4I+3
        return ap.rearrange("(p q) w -> p (q w)", q=4)

    def view(tile, off, free_ap):
        # free-dim view of a [128, W] tile: keep the partition dim, replace
        # the free dims; offset in elements from the tile base.
        base = tile[:]
        return bass.AP(tile.tensor, base.offset + off,
                       [list(base.ap[0])] + free_ap)

    P1 = [[2, 256], [-1, 2]]     # j -> j^1 (offset +1)
    P3 = [[4, 128], [-1, 4]]     # j -> j^3 (offset +3)
    PAIR = [[4, 128], [1, 2]]    # elements {4t+off, 4t+off+1}

    with TileContext(nc) as tc:
        with tc.tile_pool(name="img", bufs=3) as img_pool, \
             tc.tile_pool(name="outp", bufs=3) as out_pool, \
             tc.tile_pool(name="eo", bufs=2) as eo_pool:

            v = nc.vector
            act = nc.scalar

            X = [None] * _C
            L = [None] * _C
            D = [None] * _C

            def load(c):
                # Each channel loads as two 512 KiB halves with separate
                # completion sems: rows {4I,4I+1} then {4I+2,4I+3}.  EI only
                # needs the first half, and the DMA completion receipt lags
                # the last byte by ~2us — finer sem granularity starts each
                # channel's compute chain ~1.4us earlier, keeping ACT (and
                # so the store dispatches) ahead of the ring.
                X[c] = (img_pool.tile([128, 2048], fp32, tag="X",
                                      name=f"X{c}"), 0)
                src = img4(xt[c])
                nc.sync.dma_start(out=X[c][0][:, 0:1024], in_=src[:, 0:1024])
                nc.sync.dma_start(out=X[c][0][:, 1024:2048],
                                  in_=src[:, 1024:2048])

            def compute(c):
                EI = eo_pool.tile([128, 512], fp32, tag="EI", name=f"EI{c}")
                OIu = eo_pool.tile([128, 512], fp32, tag="OIu", name=f"OIu{c}")
                OI2 = eo_pool.tile([128, 512], fp32, tag="OI2", name=f"OI2{c}")
                Xt, xb = X[c]
                # EI = X_r0 + P1(X_r1)
                v.tensor_tensor(out=EI[:], in0=Xt[:, xb:xb + 512],
                                in1=view(Xt, xb + 512 + 1, P1), op=A.add)
                # OIu = P2(OI) = X_r2[j^2] + X_r3[j^3], built pair-split so
                # every AP stays 3D; then OI2 = 0.25*OIu.
                for h in (0, 2):
                    v.tensor_tensor(
                        out=view(OIu, h, PAIR),
                        in0=view(Xt, xb + 1024 + (h ^ 2), PAIR),
                        in1=view(Xt, xb + 1536 + (h ^ 2) + 1,
                                 [[4, 128], [-1, 2]]),
                        op=A.add)
                v.tensor_scalar_mul(OI2[:], OIu[:], 0.25)

                L[c] = out_pool.tile([128, 2048], fp32, tag="L", name=f"L{c}")
                D[c] = out_pool.tile([128, 2048], fp32, tag="D", name=f"D{c}")
                # L0/D0 into the r=0 block — fully contiguous fused ops.
                v.scalar_tensor_tensor(out=L[c][:, 0:512], in0=EI[:],
                                       scalar=0.25, in1=OI2[:],
                                       op0=A.mult, op1=A.add)
                v.scalar_tensor_tensor(out=D[c][:, 0:512], in0=EI[:],
                                       scalar=0.25, in1=OI2[:],
                                       op0=A.mult, op1=A.subtract)

                # low[r] = P_r(L0), det[r] = P_r(D0): permuted copies, all on
                # ACT (runs them at full rate; 4.9us/channel sustains the
                # 5.06us/channel store cadence).  L copies first so the L
                # store dispatches early.
                for t in (L[c], D[c]):
                    act.copy(t[:, 512:1024], view(t, 1, P1))
                    act.copy(view(t, 1024 + 0, PAIR), view(t, 2, PAIR))
                    act.copy(view(t, 1024 + 2, PAIR), view(t, 0, PAIR))
                    act.copy(t[:, 1536:2048], view(t, 3, P3))

            def store(c):
                nc.sync.dma_start(out=img4(ot[27 + c]), in_=L[c][:])
                nc.sync.dma_start(out=img4(ot[c]), in_=D[c][:])

            load(0)
            load(1)
            load(2)
            compute(0)
            store(0)
            compute(1)
            store(1)
            compute(2)
            store(2)

    nc.finalize()
    return nc


def _get_nc():
    if "nc" not in _nc_cache:
        _nc_cache["nc"] = _build_nc()
    return _nc_cache["nc"]


def run_spmd(x, **kwargs):
    """Run the SPMD kernel on 8 cores; returns (stacked_output, BassKernelResults)."""
    from concourse.bass_utils import run_bass_kernel_spmd

    x = np.ascontiguousarray(np.asarray(x, dtype=np.float32))
    assert x.shape == (_NCORES, _C, _H, _W), x.shape
    nc = _get_nc()
    in_maps = [{"x": np.ascontiguousarray(x[b])} for b in range(_NCORES)]
    res = run_bass_kernel_spmd(nc, in_maps, core_ids=list(range(_NCORES)),
                               **kwargs)
    out = np.stack([res.results[b]["out"] for b in range(_NCORES)], axis=0)
    # channels 3..26 are mathematically zero; the device relies on the
    # pre-zeroed output contract — re-assert host-side for safety.
    out[:, 3:27] = 0.0
    return out, res


def kernel(x):
    out, _ = run_spmd(x)
    return out
